# revision 2
# baseline (speedup 1.0000x reference)
"""Bass/Tile kernel for nn_EncoderBlock (dense transformer w/ graph-masked
attention + GIN MLP). Per-core program: 2 batches, L=512, C=512, H=4, HS=128,
HID=2048. Data-parallel over batch across 8 cores, no collectives.

Layout strategy (per batch):
  - LN stats in token-major (bn_stats), center/scale via tensor_scalar,
    PE-transpose to channel-major, fuse ln gamma/beta into the transpose
    copyback (per-partition scalars there).
  - qT,kT channel-major [C,L]; v token-major [L,C]  (both straight matmuls
    from xn1T, no extra transposes; per-head slices are single tiles).
  - scores computed TRANSPOSED: scoreT[lk,lq] = kT_chunk.T @ qT. Mask applied
    as a -1e30 bias accumulated into score PSUM via (-1e30*I) @ comp matmul.
    exp via ACT straight from PSUM (scale=1/sqrt(HS) folded in). Softmax
    denominator via ones-lhsT matmuls; normalization fused into the
    attention-output PSUM->SBUF copyback with a partition_broadcast recip.
  - attn-out matmuls need no transposes: lhsT = v token-major chunks.
  - proj produces y token-major directly (lhsT = OT chunks), residual fused
    into copyback.
  - GIN: g token-major (lhsT=xn2T), hT=fc1+z computed hid-major with z
    matmuls (lhsT=g chunks, rhs=adjT/adj) accumulated into the same PSUM
    bank as fc1, relu on copyback. fc2 from hT (lhsT) + residual on copyback.
  - masks: a = (|rel_pos-5|==4) in bf16; m2=aTa+I, m3=aaT+I via bf16 matmuls
    with identity-strip accumulation; binarized complements via is_lt.
    Transposed masks are free: compT(head0)=comp1, compT(head1)=comp0,
    heads 2,3 symmetric.
"""

import sys
for _p in ("/opt/trn_rl_repo", "/root/.axon_site/_ro/trn_rl_repo"):
    if _p not in sys.path:
        sys.path.append(_p)

from contextlib import ExitStack

import concourse.bass as bass
import concourse.tile as tile
from concourse import mybir
from concourse.bass import ts
from concourse.masks import make_identity

F32 = mybir.dt.float32
F32R = mybir.dt.float32r
BF16 = mybir.dt.bfloat16
OP = mybir.AluOpType
ACT = mybir.ActivationFunctionType

P = 128
L = 512
C = 512
H = 4
HS = 128
HID = 2048
NB = 2          # batches per core
LC = L // P     # 4 token chunks
CC = C // P     # 4 channel chunks
HC = HID // P   # 16 hidden chunks
EPS = 1e-5
INV_SQRT_HS = 1.0 / (HS ** 0.5)
NEG = -1e30


def build_encoder_program(nc):
    """Emit the full 2-batch encoder program into `nc`."""
    def dram(name, shape, kind):
        return nc.dram_tensor(name, shape, F32, kind=kind).ap()

    x_d = dram("x", [NB, L, C], "ExternalInput")
    rp_d = dram("rel_pos", [NB, L, L], "ExternalInput")
    adj_d = dram("adj", [NB, L, L], "ExternalInput")
    wqkv_d = dram("w_qkv", [C, 3 * C], "ExternalInput")
    wproj_d = dram("w_proj", [C, C], "ExternalInput")
    ln1g_d = dram("ln1_g", [C], "ExternalInput")
    ln1b_d = dram("ln1_b", [C], "ExternalInput")
    ln2g_d = dram("ln2_g", [C], "ExternalInput")
    ln2b_d = dram("ln2_b", [C], "ExternalInput")
    wfc1_d = dram("w_fc1", [C, HID], "ExternalInput")
    wgcn_d = dram("w_gcn", [C, HID], "ExternalInput")
    wfc2_d = dram("w_fc2", [HID, C], "ExternalInput")
    out_d = dram("out", [NB, L, C], "ExternalOutput")

    x_t3 = [x_d[b].rearrange("(lo p) c -> p lo c", p=P) for b in range(NB)]
    rp_t3 = [rp_d[b].rearrange("(lo p) c -> p lo c", p=P) for b in range(NB)]
    adj_t3 = [adj_d[b].rearrange("(lo p) c -> p lo c", p=P) for b in range(NB)]
    out_t3 = [out_d[b].rearrange("(lo p) c -> p lo c", p=P) for b in range(NB)]

    with ExitStack() as top:
        tc = top.enter_context(tile.TileContext(nc))
        const = top.enter_context(tc.tile_pool(name="const", bufs=1))
        persist = top.enter_context(tc.tile_pool(name="persist", bufs=1))
        psum = top.enter_context(tc.tile_pool(name="psum", bufs=1, space="PSUM"))

        def pmm():
            return psum.tile([P, 512], F32, tag="mm", bufs=4, name="pmm")

        def ptp(dt):
            return psum.tile([P, P], dt, tag="tp", bufs=3, name="ptp")

        # ---------------- constants ----------------
        ident_f = const.tile([P, P], F32)
        make_identity(nc, ident_f[:])
        ident_r = const.tile([P, P], F32R)
        nc.vector.tensor_copy(out=ident_r[:], in_=ident_f[:])
        ident_b = const.tile([P, P], BF16)
        nc.vector.tensor_copy(out=ident_b[:], in_=ident_f[:])
        negI_b = const.tile([P, P], BF16)
        nc.gpsimd.memset(negI_b[:], 0.0)
        nc.gpsimd.affine_select(out=negI_b[:], in_=negI_b[:],
                                compare_op=OP.not_equal, fill=NEG,
                                base=0, pattern=[[-1, P]], channel_multiplier=1)
        ones_f = const.tile([P, 1], F32)
        nc.vector.memset(ones_f[:], 1.0)
        ones_r = const.tile([P, 1], F32R)
        nc.vector.tensor_copy(out=ones_r[:], in_=ones_f[:])
        eps_t = const.tile([P, 1], F32)
        nc.vector.memset(eps_t[:], EPS)
        neg5_t = const.tile([P, 1], F32)
        nc.vector.memset(neg5_t[:], -5.0)
        ln1g = const.tile([P, CC], F32)
        nc.sync.dma_start(out=ln1g[:], in_=ln1g_d.rearrange("(ko p) -> p ko", p=P))
        ln1b = const.tile([P, CC], F32)
        nc.sync.dma_start(out=ln1b[:], in_=ln1b_d.rearrange("(ko p) -> p ko", p=P))
        ln2g = const.tile([P, CC], F32)
        nc.sync.dma_start(out=ln2g[:], in_=ln2g_d.rearrange("(ko p) -> p ko", p=P))
        ln2b = const.tile([P, CC], F32)
        nc.sync.dma_start(out=ln2b[:], in_=ln2b_d.rearrange("(ko p) -> p ko", p=P))

        # x1 residual stream (kept across phases)
        x1 = [persist.tile([P, LC, C], F32, name=f"x1_{b}", tag=f"x1_{b}")
              for b in range(NB)]

        # ---------------- layernorm helper ----------------
        def layer_norm_T(pool, xin, g_sb, b_sb, tag):
            """xin: [P, LC, C] token-major F32. Returns xnT [P, CC, L] F32R
            with gamma/beta applied (fused into the transpose copyback)."""
            xnT = pool.tile([P, CC, L], F32R, tag=f"xnT_{tag}", name="xnT")
            for i in range(LC):
                st6 = pool.tile([P, 6], F32, tag="ln_st6", bufs=2, name="st6")
                nc.vector.bn_stats(out=st6[:], in_=xin[:, i, :])
                mv = pool.tile([P, 2], F32, tag="ln_mv", bufs=2, name="mv")
                nc.vector.bn_aggr(out=mv[:], in_=st6[:])
                istd = pool.tile([P, 1], F32, tag="ln_istd", bufs=2, name="istd")
                nc.scalar.activation(out=istd[:], in_=mv[:, 1:2], func=ACT.Sqrt,
                                     bias=eps_t[:], scale=1.0)
                nc.vector.reciprocal(out=istd[:], in_=istd[:])
                xc = pool.tile([P, C], F32R, tag="ln_xc", bufs=2, name="xc")
                nc.vector.tensor_scalar(out=xc[:], in0=xin[:, i, :],
                                        scalar1=mv[:, 0:1], scalar2=istd[:],
                                        op0=OP.subtract, op1=OP.mult)
                for j in range(CC):      # channel chunk (partition of output)
                    pt = ptp(F32R)
                    nc.tensor.transpose(pt[:], xc[:, ts(j, P)], ident_r[:])
                    nc.vector.tensor_scalar(out=xnT[:, j, ts(i, P)],
                                            in0=pt[:].bitcast(F32),
                                            scalar1=g_sb[:, j:j + 1],
                                            scalar2=b_sb[:, j:j + 1],
                                            op0=OP.mult, op1=OP.add)
            return xnT

        # ================= attention phases =================
        with ExitStack() as attn_stack:
            wA = attn_stack.enter_context(tc.tile_pool(name="wA", bufs=1))
            ap = attn_stack.enter_context(tc.tile_pool(name="attn", bufs=1))

            wq = wA.tile([P, CC, 3 * C], F32R)
            nc.sync.dma_start(
                out=wq[:], in_=wqkv_d.rearrange("(ko p) n -> p ko n", p=P).bitcast(F32R))
            wp = wA.tile([P, CC, C], F32R)
            nc.sync.dma_start(
                out=wp[:], in_=wproj_d.rearrange("(ko p) n -> p ko n", p=P).bitcast(F32R))

            # eye strips (bf16): eye_b[:, i, j] = 1 iff j == 128*i + p
            eye_b = ap.tile([P, LC, L], BF16, tag="eye_b", name="eye_b")
            nc.gpsimd.memset(eye_b[:], 0.0)
            eyec_b = ap.tile([P, LC, L], BF16, tag="eyec_b", name="eyec_b")
            nc.gpsimd.memset(eyec_b[:], 1.0)
            for i in range(LC):
                nc.gpsimd.affine_select(out=eye_b[:, i, :], in_=eye_b[:, i, :],
                                        compare_op=OP.not_equal, fill=1.0,
                                        base=P * i, pattern=[[-1, L]],
                                        channel_multiplier=1)
                nc.gpsimd.affine_select(out=eyec_b[:, i, :], in_=eyec_b[:, i, :],
                                        compare_op=OP.not_equal, fill=0.0,
                                        base=P * i, pattern=[[-1, L]],
                                        channel_multiplier=1)

            for b in range(NB):
                # ---- inputs ----
                x_t = ap.tile([P, LC, C], F32, tag="x_t", bufs=2, name="x_t")
                nc.sync.dma_start(out=x_t[:], in_=x_t3[b])
                rel = ap.tile([P, LC, L], F32, tag="rel", name="rel")
                nc.sync.dma_start(out=rel[:], in_=rp_t3[b])

                # ---- hop mask: a = (|rel-5| == 4) ----
                a_b = ap.tile([P, LC, L], BF16, tag="a_b", name="a_b")
                comp0 = ap.tile([P, LC, L], BF16, tag="comp0", name="comp0")
                for i in range(LC):
                    tabs = ap.tile([P, L], F32, tag="tabs", bufs=2, name="tabs")
                    nc.scalar.activation(out=tabs[:], in_=rel[:, i, :],
                                         func=ACT.Abs, bias=neg5_t[:], scale=1.0)
                    nc.vector.tensor_scalar(out=a_b[:, i, :], in0=tabs[:],
                                            scalar1=4.0, scalar2=None,
                                            op0=OP.is_equal)
                    na = ap.tile([P, L], BF16, tag="na", bufs=2, name="na")
                    nc.vector.tensor_scalar(out=na[:], in0=tabs[:],
                                            scalar1=4.0, scalar2=None,
                                            op0=OP.not_equal)
                    nc.vector.tensor_tensor(out=comp0[:, i, :], in0=na[:],
                                            in1=eyec_b[:, i, :], op=OP.mult)
                # aT + comp1
                aT_b = ap.tile([P, LC, L], BF16, tag="aT_b", name="aT_b")
                for i in range(LC):
                    for j in range(LC):
                        pt = ptp(BF16)
                        nc.tensor.transpose(pt[:], a_b[:, i, ts(j, P)], ident_b[:])
                        nc.vector.tensor_copy(out=aT_b[:, j, ts(i, P)], in_=pt[:])
                comp1 = ap.tile([P, LC, L], BF16, tag="comp1", name="comp1")
                for i in range(LC):
                    na = ap.tile([P, L], BF16, tag="na", bufs=2, name="na")
                    nc.vector.tensor_scalar(out=na[:], in0=aT_b[:, i, :],
                                            scalar1=0.5, scalar2=None,
                                            op0=OP.is_lt)
                    nc.vector.tensor_tensor(out=comp1[:, i, :], in0=na[:],
                                            in1=eyec_b[:, i, :], op=OP.mult)
                # m2 = aT@a + I, m3 = a@aT + I -> complement masks
                comp2 = ap.tile([P, LC, L], BF16, tag="comp2", name="comp2")
                comp3 = ap.tile([P, LC, L], BF16, tag="comp3", name="comp3")
                for (cm, src) in ((comp2, a_b), (comp3, aT_b)):
                    for m in range(LC):
                        pm = pmm()
                        for k in range(LC):
                            nc.tensor.matmul(pm[:], src[:, k, ts(m, P)],
                                             src[:, k, :],
                                             start=(k == 0), stop=False)
                        nc.tensor.matmul(pm[:], ident_b[:], eye_b[:, m, :],
                                         start=False, stop=True)
                        nc.vector.tensor_scalar(out=cm[:, m, :], in0=pm[:],
                                                scalar1=0.5, scalar2=None,
                                                op0=OP.is_lt)

                # ---- LN1 -> xn1T ----
                xn1T = layer_norm_T(ap, x_t, ln1g, ln1b, "1")

                # ---- qT, kT (channel-major), v (token-major) ----
                qT = ap.tile([P, CC, L], F32R, tag="qT", name="qT")
                kT = ap.tile([P, CC, L], F32R, tag="kT", name="kT")
                for dst, off in ((qT, 0), (kT, C)):
                    for m in range(CC):
                        pm = pmm()
                        for k in range(CC):
                            nc.tensor.matmul(pm[:], wq[:, k, off + m * P:off + (m + 1) * P],
                                             xn1T[:, k, :],
                                             start=(k == 0), stop=(k == CC - 1))
                        nc.vector.tensor_copy(out=dst[:, m, :], in_=pm[:])
                v_sb = ap.tile([P, LC, C], F32R, tag="v_sb", name="v_sb")
                for m in range(LC):
                    pm = pmm()
                    for k in range(CC):
                        nc.tensor.matmul(pm[:], xn1T[:, k, ts(m, P)],
                                         wq[:, k, 2 * C:3 * C],
                                         start=(k == 0), stop=(k == CC - 1))
                    nc.vector.tensor_copy(out=v_sb[:, m, :], in_=pm[:])

                # ---- attention heads ----
                compT = [comp1, comp0, comp2, comp3]
                OT = ap.tile([P, H, L], F32R, tag="OT", name="OT")
                for h in range(H):
                    attnT = ap.tile([P, LC, L], F32R, tag="attnT", bufs=2,
                                    name="attnT")
                    for i in range(LC):
                        pm = pmm()
                        nc.tensor.matmul(pm[:], kT[:, h, ts(i, P)], qT[:, h, :],
                                         start=True, stop=False)
                        nc.tensor.matmul(pm[:], negI_b[:], compT[h][:, i, :],
                                         start=False, stop=True)
                        nc.scalar.activation(out=attnT[:, i, :], in_=pm[:],
                                             func=ACT.Exp, scale=INV_SQRT_HS)
                    pd = psum.tile([1, L], F32, tag="dn", bufs=1, name="pd")
                    for i in range(LC):
                        nc.tensor.matmul(pd[:], ones_r[:], attnT[:, i, :],
                                         start=(i == 0), stop=(i == LC - 1))
                    recip = ap.tile([1, L], F32, tag="recip", bufs=2, name="recip")
                    nc.vector.reciprocal(out=recip[:], in_=pd[:])
                    rbc = ap.tile([P, L], F32, tag="rbc", bufs=2, name="rbc")
                    nc.gpsimd.partition_broadcast(rbc[:], recip[:])
                    po = pmm()
                    for i in range(LC):
                        nc.tensor.matmul(po[:], v_sb[:, i, ts(h, P)], attnT[:, i, :],
                                         start=(i == 0), stop=(i == LC - 1))
                    nc.vector.tensor_tensor(out=OT[:, h, :], in0=po[:], in1=rbc[:],
                                            op=OP.mult)

                # ---- proj + residual -> x1 ----
                for m in range(LC):
                    pm = pmm()
                    for k in range(CC):
                        nc.tensor.matmul(pm[:], OT[:, k, ts(m, P)], wp[:, k, :],
                                         start=(k == 0), stop=(k == CC - 1))
                    nc.vector.tensor_tensor(out=x1[b][:, m, :], in0=x_t[:, m, :],
                                            in1=pm[:], op=OP.add)

        # ================= GIN phases =================
        with ExitStack() as gin_stack:
            wB = gin_stack.enter_context(tc.tile_pool(name="wB", bufs=1))
            gp = gin_stack.enter_context(tc.tile_pool(name="gin", bufs=1))

            wgc = wB.tile([P, CC, HID], F32R)
            nc.sync.dma_start(
                out=wgc[:], in_=wgcn_d.rearrange("(ko p) n -> p ko n", p=P).bitcast(F32R))
            wf1 = wB.tile([P, CC, HID], F32R)
            nc.sync.dma_start(
                out=wf1[:], in_=wfc1_d.rearrange("(ko p) n -> p ko n", p=P).bitcast(F32R))
            wf2_r = wB.tile([P, HC, C], F32R)
            nc.sync.dma_start(
                out=wf2_r[:], in_=wfc2_d.rearrange("(ko p) n -> p ko n", p=P).bitcast(F32R))

            for b in range(NB):
                # ---- adj load, cast, transpose ----
                adj_b = gp.tile([P, LC, L], BF16, tag="adj_b", name="adj_b")
                for i in range(LC):
                    stg = gp.tile([P, L], F32, tag="stage", bufs=2, name="stg")
                    nc.sync.dma_start(out=stg[:], in_=adj_t3[b][:, i, :])
                    nc.vector.tensor_copy(out=adj_b[:, i, :], in_=stg[:])
                adjT_b = gp.tile([P, LC, L], BF16, tag="adjT_b", name="adjT_b")
                for i in range(LC):
                    for j in range(LC):
                        pt = ptp(BF16)
                        nc.tensor.transpose(pt[:], adj_b[:, i, ts(j, P)], ident_b[:])
                        nc.vector.tensor_copy(out=adjT_b[:, j, ts(i, P)], in_=pt[:])

                # ---- LN2 -> xn2T ----
                xn2T = layer_norm_T(gp, x1[b], ln2g, ln2b, "2")

                # ---- g = xn2 @ w_gcn (token-major, bf16) ----
                g_b = gp.tile([P, LC, HID], BF16, tag="g_b", name="g_b")
                for m in range(LC):
                    for n in range(HID // 512):
                        pm = pmm()
                        for k in range(CC):
                            nc.tensor.matmul(pm[:], xn2T[:, k, ts(m, P)],
                                             wgc[:, k, ts(n, 512)],
                                             start=(k == 0), stop=(k == CC - 1))
                        nc.vector.tensor_copy(out=g_b[:, m, ts(n, 512)], in_=pm[:])

                # ---- hT = relu(fc1 + [adj@g1; adjT@g2])^T  (hid-major) ----
                hT_r = gp.tile([P, HC, L], F32R, tag="hT_r", name="hT_r")
                for mh in range(HC):
                    pm = pmm()
                    for k in range(CC):
                        nc.tensor.matmul(pm[:], wf1[:, k, ts(mh, P)], xn2T[:, k, :],
                                         start=(k == 0), stop=False)
                    rhs = adjT_b if mh < HC // 2 else adj_b
                    for k in range(LC):
                        nc.tensor.matmul(pm[:], g_b[:, k, ts(mh, P)], rhs[:, k, :],
                                         start=False, stop=(k == LC - 1))
                    nc.scalar.activation(out=hT_r[:, mh, :], in_=pm[:], func=ACT.Relu)

                # ---- out = x1 + hT.T @ w_fc2 ----
                for m in range(LC):
                    pm = pmm()
                    for k in range(HC):
                        nc.tensor.matmul(pm[:], hT_r[:, k, ts(m, P)], wf2_r[:, k, :],
                                         start=(k == 0), stop=(k == HC - 1))
                    o_sb = gp.tile([P, C], F32, tag="o_sb", bufs=2, name="o_sb")
                    nc.vector.tensor_tensor(out=o_sb[:], in0=x1[b][:, m, :],
                                            in1=pm[:], op=OP.add)
                    nc.sync.dma_start(out=out_t3[b][:, m, :], in_=o_sb[:])


# ======================= SPMD wrapper =======================
import numpy as np

N_CORES = 8
_CACHE = {}


def _get_program():
    if "nc" not in _CACHE:
        from concourse import bacc
        nc = bacc.Bacc("TRN2", target_bir_lowering=False, debug=False,
                       num_devices=N_CORES)
        build_encoder_program(nc)
        nc.finalize()
        _CACHE["nc"] = nc
    return _CACHE["nc"]


def kernel(**inputs):
    """Full-input entry point: shards batch dim over 8 NeuronCores,
    runs the Bass program, gathers the full output."""
    from concourse.bass_utils import run_bass_kernel_spmd

    nc = _get_program()
    B = inputs["x"].shape[0]
    assert B == NB * N_CORES, f"expected B={NB * N_CORES}, got {B}"
    shared = {k: np.ascontiguousarray(np.asarray(v, np.float32))
              for k, v in inputs.items() if k not in ("x", "rel_pos", "adj")}
    in_maps = []
    for c in range(N_CORES):
        sl = slice(NB * c, NB * (c + 1))
        m = dict(shared)
        for k in ("x", "rel_pos", "adj"):
            m[k] = np.ascontiguousarray(np.asarray(inputs[k], np.float32)[sl])
        in_maps.append(m)
    res = run_bass_kernel_spmd(nc, in_maps, list(range(N_CORES)))
    return np.concatenate([res.results[c]["out"] for c in range(N_CORES)], axis=0)
